# revision 37
# baseline (speedup 1.0000x reference)
"""BiLSTM + mean-field CRF on 8 Trainium2 NeuronCores.

Strategy: the single 16384-long sequence is split into 8 contiguous
2048-position core slices (data-parallel across cores). Inside each core the
sequence is further split into 128 lanes of 17 consecutive positions each;
every lane warm-starts K=10 steps early from zero state (LSTM forget gates
=> state influence decays geometrically, truncation error ~3e-3 after the
CRF). Both LSTM directions run as batched 128-lane recurrences; gates come
from one fused PSUM accumulation per quarter: a K=1 bias matmul whose lhsT
doubles as the lane-validity mask (lanes before sequence start see zero
bias + zero x + zero h and hence stay exactly at zero state), four bf16
x-chunk matmuls, and two fp8e4 DoubleRow h-chunk matmuls (K=256 each --
measured 2x bf16 throughput; h-part-only fp8 keeps the end-to-end error at
~9e-3, x-part must stay bf16). The hidden state is re-transposed each step
with TensorE transpose; the transposed copy is kept in both bf16 (logits)
and fp8 (next step's stationary operand). Logits are computed per step with
tiny N=32 matmuls and scattered to DRAM in position order; the CRF (conv
kernel as a banded 128x128 symmetric Toeplitz matmul applied to all 28
position tiles in two wide matmuls, softmax via free-dim reduce in a
position-on-partitions layout) runs on position tiles of 128 at stride 78
whose edges erode 5 positions per iteration.
"""
import sys

sys.path.insert(0, "/opt/trn_rl_repo")

import numpy as np
import ml_dtypes

import concourse.bass as bass
import concourse.bacc as bacc
import concourse.mybir as mybir
from concourse.tile import TileContext
from concourse.bass_utils import run_bass_kernel_spmd

F32 = mybir.dt.float32
BF16 = mybir.dt.bfloat16
FP8 = mybir.dt.float8e4
PM8 = mybir.MatmulPerfMode.DoubleRow
AF = mybir.ActivationFunctionType

SEQ, EMB, H, G, C = 16384, 512, 512, 2048, 32
NCORES = 8
K = 8                  # halo warm-up steps
ST = 17                # positions per lane
NL = 128               # lanes
STEPS = K + ST         # 27
WINW = NL * ST         # 2176
XW = K + WINW          # 2186 x-window columns per k-tile
CST, NT = 78, 28       # CRF tile stride / count
CRFW = NT * C          # 896
LOGR = 2304            # logits scratch rows
OUTR = 2240            # output rows per core
FILT, NIT = 11, 5

_CACHE = {}


def _build():
    nc = bacc.Bacc("TRN2", target_bir_lowering=False, debug=False, num_devices=NCORES)

    def din(name, shape, dt=BF16):
        return nc.dram_tensor(name, shape, dt, kind="ExternalInput")

    # emission order = DMA priority order: fwd step-0/1 deps first
    biasf = din("biasf", [1, G])
    maskf = din("maskf", [1, STEPS * NL])
    xtf = din("xtf", [4, 128, XW])
    wihf = din("wihf", [4, 128, G])
    whhf8 = din("whhf8", [4, 128, G], FP8)
    biasb = din("biasb", [1, G])
    maskb = din("maskb", [1, STEPS * NL])
    xtb = din("xtb", [4, 128, XW])
    wihb = din("wihb", [4, 128, G])
    whhb8 = din("whhb8", [4, 128, G], FP8)
    wlinf = din("wlinf", [4, 128, C])
    wlinb = din("wlinb", [4, 128, C])
    blin = din("blin", [1, C])
    ones = din("ones", [1, 128])
    ident = din("ident", [128, 128])
    rmat = din("rmat", [128, 128], F32)
    shi = din("shi", [128, 128])
    slo = din("slo", [128, 128])
    valid = din("valid", [128, NT], F32)

    out = nc.dram_tensor("out", [OUTR, C], F32, kind="ExternalOutput")
    logf_d = nc.dram_tensor("logf_d", [LOGR, C], F32)
    logb_d = nc.dram_tensor("logb_d", [LOGR, C], F32)

    with TileContext(nc) as tc:
        with (
            tc.tile_pool(name="consts", bufs=1) as cp,
            tc.tile_pool(name="state", bufs=2) as sp,
        ):
            # ---- load constants/inputs into SBUF (DMA priority order) ----
            xt_sb, wih_sb, whh_sb, bias_sb, msk_sb, wlin_sb = {}, {}, {}, {}, {}, {}
            for d, (xs, wi, wh, bs, ms) in (
                ("f", (xtf, wihf, whhf8, biasf, maskf)),
                ("b", (xtb, wihb, whhb8, biasb, maskb)),
            ):
                # bias/mask live in row 0 of K=128 operands (rows 1-127 zero)
                # so the bias inject is a full-K matmul (K=1 matmuls are slow).
                t = cp.tile([128, G], BF16, name=f"bias{d}")
                nc.vector.memset(t[:], 0.0)
                nc.sync.dma_start(out=t[0:1, :], in_=bs[:])
                bias_sb[d] = t
                t = cp.tile([128, STEPS * NL], BF16, name=f"msk{d}")
                nc.vector.memset(t[:], 0.0)
                nc.sync.dma_start(out=t[0:1, :], in_=ms[:])
                msk_sb[d] = t
                t = cp.tile([128, 4 * XW], BF16, name=f"xt{d}")
                for k in range(4):
                    for h0 in (0, 64):
                        nc.sync.dma_start(out=t[h0:h0 + 64, k * XW:(k + 1) * XW],
                                          in_=xs[k][h0:h0 + 64])
                xt_sb[d] = t
                t = cp.tile([128, 4 * G], BF16, name=f"wih{d}")
                for k in range(4):
                    for h0 in (0, 64):
                        nc.sync.dma_start(out=t[h0:h0 + 64, k * G:(k + 1) * G],
                                          in_=wi[k][h0:h0 + 64])
                wih_sb[d] = t
                t = cp.tile([128, 4 * G], FP8, name=f"whh{d}")
                for k in range(4):
                    nc.sync.dma_start(out=t[:, k * G:(k + 1) * G], in_=wh[k])
                whh_sb[d] = t
            for d, wl in (("f", wlinf), ("b", wlinb)):
                t = cp.tile([128, 4 * C], BF16, name=f"wlin{d}")
                for k in range(4):
                    nc.sync.dma_start(out=t[:, k * C:(k + 1) * C], in_=wl[k])
                wlin_sb[d] = t
            blin_sb = cp.tile([1, C], BF16, name="blin")
            nc.sync.dma_start(out=blin_sb[:], in_=blin[:])
            ones_sb = cp.tile([1, 128], BF16, name="ones")
            nc.sync.dma_start(out=ones_sb[:], in_=ones[:])
            id_sb = cp.tile([128, 128], BF16, name="ident")
            nc.sync.dma_start(out=id_sb[:], in_=ident[:])
            r_sb = cp.tile([128, 128], F32, name="rmat")
            nc.sync.dma_start(out=r_sb[:], in_=rmat[:])
            logit_sb = {
                "f": cp.tile([128, ST * C], F32, name="logitf"),
                "b": cp.tile([128, ST * C], F32, name="logitb"),
            }

            # ---- recurrence ----
            lstm_psum = tc.tile_pool(name="psg", bufs=4, space="PSUM")
            pg = lstm_psum.__enter__()
            lstm_psum2 = tc.tile_pool(name="pst", bufs=2, space="PSUM")
            pt = lstm_psum2.__enter__()
            lstm_psum3 = tc.tile_pool(name="psl", bufs=2, space="PSUM")
            pl = lstm_psum3.__enter__()
            cprev, hT8prev, gates = {}, {}, {}

            FUNCS = [AF.Sigmoid, AF.Sigmoid, AF.Tanh, AF.Sigmoid]

            def emit_quarters(d, t):
                g4 = []
                mrow = msk_sb[d][:, t * NL:(t + 1) * NL]
                for q in range(4):
                    ps = pg.tile([128, 512], F32, name=f"ps{d}{t}{q}", tag="gq")
                    nc.tensor.matmul(ps[:], lhsT=mrow,
                                     rhs=bias_sb[d][:, 512 * q:512 * (q + 1)],
                                     start=True, stop=False)
                    # (mask row 0 x bias row 0: full-K matmul, rows 1-127 zero)
                    for k in range(4):
                        lhsT = xt_sb[d][:, k * XW + t: k * XW + t + ST * (NL - 1) + 1: ST]
                        nc.tensor.matmul(ps[:], lhsT=lhsT,
                                         rhs=wih_sb[d][:, k * G + 512 * q: k * G + 512 * (q + 1)],
                                         start=False, stop=(t == 0 and k == 3))
                    if t > 0:
                        w8 = whh_sb[d][:].rearrange("p (k g) -> p k g", k=4)
                        h8 = hT8prev[d][:].rearrange("p (k l) -> p k l", k=4)
                        for kp in range(2):
                            nc.tensor.matmul(
                                ps[:],
                                lhsT=h8[:, 2 * kp:2 * kp + 2, :],
                                rhs=w8[:, 2 * kp:2 * kp + 2, 512 * q:512 * (q + 1)],
                                start=False, stop=(kp == 1), perf_mode=PM8)
                    gt = sp.tile([128, 512], BF16, name=f"g{d}{t}{q}", tag=f"g{q}{d}")
                    nc.scalar.activation(gt[:], ps[:], FUNCS[q])
                    g4.append(gt)
                gates[d] = g4

            def emit_tail(d, t):
                gi, gf, gg, go = gates[d]
                cn = sp.tile([128, H], BF16, name=f"cn{d}{t}", tag=f"cn{d}")
                if t == 0:
                    nc.vector.tensor_mul(cn[:], gi[:], gg[:])
                else:
                    ig = sp.tile([128, H], BF16, name=f"ig{d}{t}", tag=f"ig{d}")
                    nc.vector.tensor_mul(ig[:], gi[:], gg[:])
                    fc = sp.tile([128, H], BF16, name=f"fc{d}{t}", tag=f"fc{d}")
                    nc.vector.tensor_mul(fc[:], gf[:], cprev[d][:])
                    nc.vector.tensor_add(cn[:], ig[:], fc[:])
                th = sp.tile([128, H], BF16, name=f"th{d}{t}", tag=f"th{d}")
                nc.scalar.activation(th[:], cn[:], AF.Tanh)
                hn = sp.tile([128, H], BF16, name=f"hn{d}{t}", tag=f"hn{d}")
                nc.vector.tensor_mul(hn[:], go[:], th[:])
                ps = pt.tile([128, H], BF16, name=f"ptr{d}{t}", tag="tr")
                for k in range(4):
                    nc.tensor.transpose(ps[:, 128 * k:128 * (k + 1)],
                                        hn[:, 128 * k:128 * (k + 1)], id_sb[:])
                hT8 = sp.tile([128, H], FP8, name=f"hT8{d}{t}", tag=f"hT8{d}")
                nc.vector.tensor_copy(hT8[:], ps[:])
                cprev[d], hT8prev[d] = cn, hT8
                if t >= K:
                    s = t - K
                    hT = sp.tile([128, H], BF16, name=f"hT{d}{t}", tag=f"hT{d}")
                    nc.vector.tensor_copy(hT[:], ps[:])
                    psl = pl.tile([128, C], F32, name=f"pl{d}{t}", tag="lg")
                    for k in range(4):
                        nc.tensor.matmul(psl[:], lhsT=hT[:, 128 * k:128 * (k + 1)],
                                         rhs=wlin_sb[d][:, C * k:C * (k + 1)],
                                         start=(k == 0),
                                         stop=(k == 3 and d == "b"))
                    if d == "f":
                        nc.tensor.matmul(psl[:], lhsT=ones_sb[:], rhs=blin_sb[:],
                                         start=False, stop=True)
                    slot = s if d == "f" else (ST - 1 - s)
                    nc.vector.tensor_copy(logit_sb[d][:, C * slot:C * (slot + 1)],
                                          psl[:])

            # the bwd dir runs D steps behind the fwd dir so the fwd
            # recurrence fills the PE while the bwd inputs still stream in
            D = 8
            for t in range(STEPS + D):
                tb = t - D
                if tb > 0:
                    emit_tail("b", tb - 1)
                if t < STEPS:
                    emit_quarters("f", t)
                if 0 <= tb < STEPS:
                    emit_quarters("b", tb)
                if t < STEPS:
                    emit_tail("f", t)
            emit_tail("b", STEPS - 1)

            # fwd logits straight to DRAM rows 17l+s
            nc.sync.dma_start(
                out=logf_d[0:WINW, :].rearrange("(l s) c -> l (s c)", s=ST),
                in_=logit_sb["f"][:],
            )
            # zero the never-written scratch tails so the CRF u-loads read 0
            zt = sp.tile([128, C], F32, name="ztail", tag="ztail")
            nc.vector.memset(zt[:], 0.0)
            nc.sync.dma_start(out=logf_d[WINW:LOGR, :], in_=zt[:])
            nc.sync.dma_start(out=logb_d[WINW:LOGR, :], in_=zt[:])
            lstm_psum3.__exit__(None, None, None)
            lstm_psum2.__exit__(None, None, None)
            lstm_psum.__exit__(None, None, None)

            # start the fwd CRF unary loads while the bwd reversal runs
            NTH = NT // 2
            CW2 = NTH * C
            uf_h = {}
            for hh in (0, 1):
                uf = sp.tile([128, CW2], F32, name=f"uf{hh}", tag=f"uf{hh}")
                nc.sync.dma_start(out=uf[:].rearrange("p (T c) -> p T c", c=C),
                                  in_=bass.AP(logf_d[:].tensor, hh * NTH * CST * C,
                                              [[C, 128], [CST * C, NTH], [1, C]]))
                uf_h[hh] = uf

            # ---- reverse bwd logits lanes (R @ logitB), then to DRAM ----
            with tc.tile_pool(name="prev", bufs=1, space="PSUM") as pr:
                psr = pr.tile([128, ST * C], F32, name="psrev")
                nc.tensor.matmul(psr[:, 0:512], lhsT=r_sb[:], rhs=logit_sb["b"][:, 0:512],
                                 start=True, stop=True)
                nc.tensor.matmul(psr[:, 512:ST * C], lhsT=r_sb[:],
                                 rhs=logit_sb["b"][:, 512:ST * C], start=True, stop=True)
                lrev = sp.tile([128, ST * C], F32, name="lrev", tag="lrev")
                nc.vector.tensor_copy(lrev[:], psr[:])
                nc.sync.dma_start(
                    out=logb_d[0:WINW, :].rearrange("(l s) c -> l (s c)", s=ST),
                    in_=lrev[:],
                )

            # ---- CRF ----
            with (
                tc.tile_pool(name="crf", bufs=2) as fp,
                tc.tile_pool(name="crfc", bufs=1) as fc1,
                tc.tile_pool(name="psc", bufs=2, space="PSUM") as pc,
            ):
                shi_sb = fc1.tile([128, 128], BF16, name="shi")
                nc.sync.dma_start(out=shi_sb[:], in_=shi[:])
                slo_sb = fc1.tile([128, 128], BF16, name="slo")
                nc.sync.dma_start(out=slo_sb[:], in_=slo[:])
                valid_sb = fc1.tile([128, NT], F32, name="valid")
                nc.sync.dma_start(out=valid_sb[:], in_=valid[:])

                # two independent half-window pipelines (T 0..13 / 14..27) so
                # the serial softmax chain of one half overlaps the other's
                # matmuls across engines
                u_h, xcur = {}, {}
                for hh in (0, 1):
                    ub = fc1.tile([128, CW2], F32, name=f"ub{hh}")
                    nc.sync.dma_start(out=ub[:].rearrange("p (T c) -> p T c", c=C),
                                      in_=bass.AP(logb_d[:].tensor,
                                                  64 * C + hh * NTH * CST * C,
                                                  [[C, 128], [CST * C, NTH], [1, C]]))
                    u = fc1.tile([128, CW2], F32, name=f"u{hh}")
                    nc.vector.tensor_add(u[:], uf_h[hh][:], ub[:])
                    u_h[hh] = u
                    xcur[hh] = u
                for it in range(NIT + 1):
                    last = it == NIT
                    e_h = {}
                    for hh in (0, 1):
                        e = fp.tile([128, CW2], F32, name=f"e{it}{hh}", tag=f"e{hh}")
                        nc.scalar.activation(e[:], xcur[hh][:], AF.Exp)
                        e_h[hh] = e
                    for hh in (0, 1):
                        e = e_h[hh]
                        ssum = fp.tile([128, NTH], F32, name=f"ss{it}{hh}", tag=f"ss{hh}")
                        nc.vector.reduce_sum(ssum[:],
                                             e[:].rearrange("p (T c) -> p T c", c=C),
                                             axis=mybir.AxisListType.X)
                        rv = fp.tile([128, NTH], F32, name=f"rv{it}{hh}", tag=f"rv{hh}")
                        nc.vector.reciprocal(rv[:], ssum[:])
                        if not last:
                            rvv = fp.tile([128, NTH], F32, name=f"rvv{it}{hh}", tag=f"rvv{hh}")
                            nc.vector.tensor_mul(rvv[:], rv[:],
                                                 valid_sb[:, hh * NTH:(hh + 1) * NTH])
                            p = fp.tile([128, CW2], BF16, name=f"p{it}{hh}", tag=f"p{hh}")
                            nc.vector.tensor_mul(
                                p[:].rearrange("p (T c) -> p T c", c=C),
                                e[:].rearrange("p (T c) -> p T c", c=C),
                                rvv[:].unsqueeze(2).broadcast_to([128, NTH, C]))
                            psc = pc.tile([128, CW2], F32, name=f"pc{it}{hh}", tag=f"pc{hh}")
                            nc.tensor.matmul(psc[:], lhsT=shi_sb[:], rhs=p[:],
                                             start=True, stop=False)
                            nc.tensor.matmul(psc[:], lhsT=slo_sb[:], rhs=p[:],
                                             start=False, stop=True)
                            xn = fp.tile([128, CW2], F32, name=f"x{it}{hh}", tag=f"x{hh}")
                            nc.vector.tensor_add(xn[:], u_h[hh][:], psc[:])
                            xcur[hh] = xn
                        else:
                            pout = fp.tile([128, CW2], F32, name=f"pout{hh}", tag=f"p{hh}")
                            nc.vector.tensor_mul(
                                pout[:].rearrange("p (T c) -> p T c", c=C),
                                e[:].rearrange("p (T c) -> p T c", c=C),
                                rv[:].unsqueeze(2).broadcast_to([128, NTH, C]))
                            nc.sync.dma_start(
                                out=bass.AP(out[:].tensor,
                                            25 * C + hh * NTH * CST * C,
                                            [[C, CST], [CST * C, NTH], [1, C]]),
                                in_=pout[25:25 + CST, :].rearrange("p (T c) -> p T c", c=C))

    nc.compile()
    return nc


def _prep(inputs):
    I = {k: np.asarray(v, np.float32) for k, v in inputs.items()}
    x = I["batch"]
    xr = x[::-1]
    bf = ml_dtypes.bfloat16
    f8 = ml_dtypes.float8_e4m3

    Wihf = I["W_ih_f"].T          # (512, 2048)
    Wihb = I["W_ih_b"].T
    Whhf = I["W_hh_f"].T          # (512, 2048)
    Whhb = I["W_hh_b"].T
    biasf = (I["b_ih_f"] + I["b_hh_f"])[None, :]
    biasb = (I["b_ih_b"] + I["b_hh_b"])[None, :]
    WlinT = I["W_lin"].T          # (1024, 32)

    half = FILT // 2
    dd = np.arange(-half, half + 1, dtype=np.float32)
    kern = np.exp(-(dd * I["inv_smoothness_theta"][0]) ** 2 / 2)
    kern[half] = 0.0
    kern *= I["smoothness_weight"]
    S = np.zeros((128, 128), np.float32)
    for i in range(128):
        for j in range(max(0, i - half), min(128, i + half + 1)):
            if i != j:
                S[i, j] = kern[j - i + half]
    S_hi = S.astype(bf).astype(np.float32)
    S_lo = (S - S_hi).astype(bf)

    shared = dict(
        wihf=Wihf.reshape(4, 128, G).astype(bf),
        wihb=Wihb.reshape(4, 128, G).astype(bf),
        whhf8=Whhf.reshape(4, 128, G).astype(f8),
        whhb8=Whhb.reshape(4, 128, G).astype(f8),
        biasf=biasf.astype(bf), biasb=biasb.astype(bf),
        wlinf=WlinT[:512].reshape(4, 128, C).astype(bf),
        wlinb=WlinT[512:].reshape(4, 128, C).astype(bf),
        blin=I["b_lin"][None, :].astype(bf),
        ones=np.ones((1, 128), bf),
        ident=np.eye(128, dtype=np.float32).astype(bf),
        rmat=np.eye(128, dtype=np.float32)[::-1].copy(),
        shi=S_hi.astype(bf), slo=S_lo,
    )

    def window(src, W0):
        w = np.zeros((K + WINW, EMB), np.float32)
        lo, hi = W0 - K, W0 + WINW
        slo, shi_ = max(lo, 0), min(hi, SEQ)
        if shi_ > slo:
            w[slo - lo:shi_ - lo] = src[slo:shi_]
        return np.ascontiguousarray(w.T).reshape(4, 128, K + WINW).astype(bf)

    st = np.arange(STEPS)[:, None]      # (STEPS, 1)
    ll = np.arange(NL)[None, :] * ST    # (1, NL)
    pp = np.arange(128)[:, None]
    TT = np.arange(NT)[None, :] * CST
    in_maps = []
    for c in range(NCORES):
        Wc = 2048 * c - 32
        Wr = 2048 * (7 - c) - 32
        gpos = Wc + TT + pp
        m = dict(shared)
        m["xtf"] = window(x, Wc)
        m["xtb"] = window(xr, Wr)
        # mask[t, lane] = 1 if that (lane, step) reads a position >= 0;
        # it is the lhsT of the bias matmul, so masked lanes get zero bias
        # and (with zeroed x and h) stay exactly at zero state.
        m["maskf"] = ((ll + st + Wc - K) >= 0).astype(bf).reshape(1, STEPS * NL)
        m["maskb"] = ((ll + st + Wr - K) >= 0).astype(bf).reshape(1, STEPS * NL)
        m["valid"] = ((gpos >= 0) & (gpos < SEQ) & (TT + pp < WINW)).astype(np.float32)
        in_maps.append(m)
    return in_maps


def _run(inputs, trace=False, trace_cores=None):
    if "nc" not in _CACHE:
        _CACHE["nc"] = _build()
    nc = _CACHE["nc"]
    in_maps = _prep(inputs)
    kw = {}
    if trace:
        import types
        try:
            import trn_agent_boot.trn_boot as tb
            hook = tb._ntff_profile_via_ctypes("/opt/axon/libaxon_pjrt.so")
            mod = types.ModuleType("antenv.axon_hooks")
            mod.get_axon_ntff_profile_hook = lambda: hook
            sys.modules.setdefault("antenv.axon_hooks", mod)
        except Exception:
            pass
        kw = dict(trace=True, trace_cores=trace_cores or list(range(NCORES)))
    res = run_bass_kernel_spmd(nc, in_maps, list(range(NCORES)), **kw)
    full = np.zeros((SEQ, C), np.float32)
    for c in range(NCORES):
        full[2048 * c:2048 * (c + 1)] = res.results[c]["out"][32:2080]
    return full, res


def kernel(**inputs):
    full, _ = _run(inputs)
    return full


# revision 38
# speedup vs baseline: 1.2154x; 1.2154x over previous
"""BiLSTM + mean-field CRF on 8 Trainium2 NeuronCores.

Strategy: the single 16384-long sequence is split into 8 contiguous
2048-position core slices (data-parallel across cores). Inside each core the
sequence is further split into 128 lanes of 17 consecutive positions each;
every lane warm-starts K=10 steps early from zero state (LSTM forget gates
=> state influence decays geometrically, truncation error ~3e-3 after the
CRF). Both LSTM directions run as batched 128-lane recurrences; gates come
from one fused PSUM accumulation per quarter: a K=1 bias matmul whose lhsT
doubles as the lane-validity mask (lanes before sequence start see zero
bias + zero x + zero h and hence stay exactly at zero state), four bf16
x-chunk matmuls, and two fp8e4 DoubleRow h-chunk matmuls (K=256 each --
measured 2x bf16 throughput; h-part-only fp8 keeps the end-to-end error at
~9e-3, x-part must stay bf16). The hidden state is re-transposed each step
with TensorE transpose; the transposed copy is kept in both bf16 (logits)
and fp8 (next step's stationary operand). Logits are computed per step with
tiny N=32 matmuls and scattered to DRAM in position order; the CRF (conv
kernel as a banded 128x128 symmetric Toeplitz matmul applied to all 28
position tiles in two wide matmuls, softmax via free-dim reduce in a
position-on-partitions layout) runs on position tiles of 128 at stride 78
whose edges erode 5 positions per iteration.
"""
import sys

sys.path.insert(0, "/opt/trn_rl_repo")

import numpy as np
import ml_dtypes

import concourse.bass as bass
import concourse.bacc as bacc
import concourse.mybir as mybir
from concourse.tile import TileContext
from concourse.bass_utils import run_bass_kernel_spmd

F32 = mybir.dt.float32
BF16 = mybir.dt.bfloat16
FP8 = mybir.dt.float8e4
PM8 = mybir.MatmulPerfMode.DoubleRow
AF = mybir.ActivationFunctionType

SEQ, EMB, H, G, C = 16384, 512, 512, 2048, 32
NCORES = 8
K = 8                  # halo warm-up steps
ST = 17                # positions per lane
NL = 128               # lanes
STEPS = K + ST         # 27
WINW = NL * ST         # 2176
XW = K + WINW          # 2186 x-window columns per k-tile
CST, NT = 78, 28       # CRF tile stride / count
CRFW = NT * C          # 896
LOGR = 2304            # logits scratch rows
OUTR = 2240            # output rows per core
FILT, NIT = 11, 5

_CACHE = {}


def _build():
    nc = bacc.Bacc("TRN2", target_bir_lowering=False, debug=False, num_devices=NCORES)

    def din(name, shape, dt=BF16):
        return nc.dram_tensor(name, shape, dt, kind="ExternalInput")

    # emission order = DMA priority order: fwd step-0/1 deps first
    biasf = din("biasf", [1, G])
    maskf = din("maskf", [1, STEPS * NL])
    xtf = din("xtf", [4, 128, XW])
    wihf = din("wihf", [4, 128, G])
    whhf8 = din("whhf8", [4, 128, G], FP8)
    biasb = din("biasb", [1, G])
    maskb = din("maskb", [1, STEPS * NL])
    xtb = din("xtb", [4, 128, XW])
    wihb = din("wihb", [4, 128, G])
    whhb8 = din("whhb8", [4, 128, G], FP8)
    wlinf = din("wlinf", [4, 128, C])
    wlinb = din("wlinb", [4, 128, C])
    blin = din("blin", [1, C])
    ones = din("ones", [1, 128])
    ident = din("ident", [128, 128])
    rmat = din("rmat", [128, 128], F32)
    shi = din("shi", [128, 128])
    slo = din("slo", [128, 128])
    valid = din("valid", [128, NT], F32)

    out = nc.dram_tensor("out", [OUTR, C], F32, kind="ExternalOutput")
    logf_d = nc.dram_tensor("logf_d", [LOGR, C], F32)
    logb_d = nc.dram_tensor("logb_d", [LOGR, C], F32)

    with TileContext(nc) as tc:
        with (
            tc.tile_pool(name="consts", bufs=1) as cp,
            tc.tile_pool(name="state", bufs=2) as sp,
        ):
            # ---- load constants/inputs into SBUF (DMA priority order) ----
            xt_sb, wih_sb, whh_sb, bias_sb, msk_sb, wlin_sb = {}, {}, {}, {}, {}, {}
            for d, (xs, wi, wh, bs, ms) in (
                ("f", (xtf, wihf, whhf8, biasf, maskf)),
                ("b", (xtb, wihb, whhb8, biasb, maskb)),
            ):
                # bias/mask live in row 0 of K=128 operands (rows 1-127 zero)
                # so the bias inject is a full-K matmul (K=1 matmuls are slow).
                t = cp.tile([128, G], BF16, name=f"bias{d}")
                nc.vector.memset(t[:], 0.0)
                nc.sync.dma_start(out=t[0:1, :], in_=bs[:])
                bias_sb[d] = t
                t = cp.tile([128, STEPS * NL], BF16, name=f"msk{d}")
                nc.vector.memset(t[:], 0.0)
                nc.sync.dma_start(out=t[0:1, :], in_=ms[:])
                msk_sb[d] = t
                t = cp.tile([128, 4 * XW], BF16, name=f"xt{d}")
                for k in range(4):
                    for h0 in (0, 64):
                        nc.sync.dma_start(out=t[h0:h0 + 64, k * XW:(k + 1) * XW],
                                          in_=xs[k][h0:h0 + 64])
                xt_sb[d] = t
                t = cp.tile([128, 4 * G], BF16, name=f"wih{d}")
                for k in range(4):
                    for h0 in (0, 64):
                        nc.sync.dma_start(out=t[h0:h0 + 64, k * G:(k + 1) * G],
                                          in_=wi[k][h0:h0 + 64])
                wih_sb[d] = t
                t = cp.tile([128, 4 * G], FP8, name=f"whh{d}")
                for k in range(4):
                    nc.sync.dma_start(out=t[:, k * G:(k + 1) * G], in_=wh[k])
                whh_sb[d] = t
            for d, wl in (("f", wlinf), ("b", wlinb)):
                t = cp.tile([128, 4 * C], BF16, name=f"wlin{d}")
                for k in range(4):
                    nc.sync.dma_start(out=t[:, k * C:(k + 1) * C], in_=wl[k])
                wlin_sb[d] = t
            blin_sb = cp.tile([1, C], BF16, name="blin")
            nc.sync.dma_start(out=blin_sb[:], in_=blin[:])
            ones_sb = cp.tile([1, 128], BF16, name="ones")
            nc.sync.dma_start(out=ones_sb[:], in_=ones[:])
            id_sb = cp.tile([128, 128], BF16, name="ident")
            nc.sync.dma_start(out=id_sb[:], in_=ident[:])
            r_sb = cp.tile([128, 128], F32, name="rmat")
            nc.sync.dma_start(out=r_sb[:], in_=rmat[:])
            logit_sb = {
                "f": cp.tile([128, ST * C], F32, name="logitf"),
                "b": cp.tile([128, ST * C], F32, name="logitb"),
            }

            # ---- recurrence ----
            lstm_psum = tc.tile_pool(name="psg", bufs=4, space="PSUM")
            pg = lstm_psum.__enter__()
            lstm_psum2 = tc.tile_pool(name="pst", bufs=2, space="PSUM")
            pt = lstm_psum2.__enter__()
            lstm_psum3 = tc.tile_pool(name="psl", bufs=2, space="PSUM")
            pl = lstm_psum3.__enter__()
            cprev, hT8prev, gates = {}, {}, {}

            FUNCS = [AF.Sigmoid, AF.Sigmoid, AF.Tanh, AF.Sigmoid]

            def emit_quarters(d, t):
                g4 = []
                mrow = msk_sb[d][:, t * NL:(t + 1) * NL]
                for q in range(4):
                    ps = pg.tile([128, 512], F32, name=f"ps{d}{t}{q}", tag="gq")
                    nc.tensor.matmul(ps[:], lhsT=mrow,
                                     rhs=bias_sb[d][:, 512 * q:512 * (q + 1)],
                                     start=True, stop=False)
                    # (mask row 0 x bias row 0: full-K matmul, rows 1-127 zero)
                    for k in range(4):
                        lhsT = xt_sb[d][:, k * XW + t: k * XW + t + ST * (NL - 1) + 1: ST]
                        nc.tensor.matmul(ps[:], lhsT=lhsT,
                                         rhs=wih_sb[d][:, k * G + 512 * q: k * G + 512 * (q + 1)],
                                         start=False, stop=(t == 0 and k == 3))
                    if t > 0:
                        w8 = whh_sb[d][:].rearrange("p (k g) -> p k g", k=4)
                        h8 = hT8prev[d][:].rearrange("p (k l) -> p k l", k=4)
                        for kp in range(2):
                            nc.tensor.matmul(
                                ps[:],
                                lhsT=h8[:, 2 * kp:2 * kp + 2, :],
                                rhs=w8[:, 2 * kp:2 * kp + 2, 512 * q:512 * (q + 1)],
                                start=False, stop=(kp == 1), perf_mode=PM8)
                    gt = sp.tile([128, 512], BF16, name=f"g{d}{t}{q}", tag=f"g{q}{d}")
                    nc.scalar.activation(gt[:], ps[:], FUNCS[q])
                    g4.append(gt)
                gates[d] = g4

            def emit_tail(d, t):
                gi, gf, gg, go = gates[d]
                cn = sp.tile([128, H], BF16, name=f"cn{d}{t}", tag=f"cn{d}")
                if t == 0:
                    nc.vector.tensor_mul(cn[:], gi[:], gg[:])
                else:
                    ig = sp.tile([128, H], BF16, name=f"ig{d}{t}", tag=f"ig{d}")
                    nc.vector.tensor_mul(ig[:], gi[:], gg[:])
                    fc = sp.tile([128, H], BF16, name=f"fc{d}{t}", tag=f"fc{d}")
                    nc.vector.tensor_mul(fc[:], gf[:], cprev[d][:])
                    nc.vector.tensor_add(cn[:], ig[:], fc[:])
                th = sp.tile([128, H], BF16, name=f"th{d}{t}", tag=f"th{d}")
                nc.scalar.activation(th[:], cn[:], AF.Tanh)
                hn = sp.tile([128, H], BF16, name=f"hn{d}{t}", tag=f"hn{d}")
                nc.vector.tensor_mul(hn[:], go[:], th[:])
                ps = pt.tile([128, H], BF16, name=f"ptr{d}{t}", tag="tr")
                for k in range(4):
                    nc.tensor.transpose(ps[:, 128 * k:128 * (k + 1)],
                                        hn[:, 128 * k:128 * (k + 1)], id_sb[:])
                hT8 = sp.tile([128, H], FP8, name=f"hT8{d}{t}", tag=f"hT8{d}")
                nc.vector.tensor_copy(hT8[:], ps[:])
                cprev[d], hT8prev[d] = cn, hT8
                if t >= K:
                    s = t - K
                    hT = sp.tile([128, H], BF16, name=f"hT{d}{t}", tag=f"hT{d}")
                    nc.vector.tensor_copy(hT[:], ps[:])
                    psl = pl.tile([128, C], F32, name=f"pl{d}{t}", tag="lg")
                    for k in range(4):
                        nc.tensor.matmul(psl[:], lhsT=hT[:, 128 * k:128 * (k + 1)],
                                         rhs=wlin_sb[d][:, C * k:C * (k + 1)],
                                         start=(k == 0),
                                         stop=(k == 3 and d == "b"))
                    if d == "f":
                        nc.tensor.matmul(psl[:], lhsT=ones_sb[:], rhs=blin_sb[:],
                                         start=False, stop=True)
                    slot = s if d == "f" else (ST - 1 - s)
                    nc.vector.tensor_copy(logit_sb[d][:, C * slot:C * (slot + 1)],
                                          psl[:])

            for t in range(STEPS):
                if t > 0:
                    emit_tail("b", t - 1)
                emit_quarters("f", t)
                emit_quarters("b", t)
                emit_tail("f", t)
            emit_tail("b", STEPS - 1)

            # fwd logits straight to DRAM rows 17l+s
            nc.sync.dma_start(
                out=logf_d[0:WINW, :].rearrange("(l s) c -> l (s c)", s=ST),
                in_=logit_sb["f"][:],
            )
            # zero the never-written scratch tails so the CRF u-loads read 0
            zt = sp.tile([128, C], F32, name="ztail", tag="ztail")
            nc.vector.memset(zt[:], 0.0)
            nc.sync.dma_start(out=logf_d[WINW:LOGR, :], in_=zt[:])
            nc.sync.dma_start(out=logb_d[WINW:LOGR, :], in_=zt[:])
            lstm_psum3.__exit__(None, None, None)
            lstm_psum2.__exit__(None, None, None)
            lstm_psum.__exit__(None, None, None)

            # start the fwd CRF unary loads while the bwd reversal runs
            NTH = NT // 2
            CW2 = NTH * C
            uf_h = {}
            for hh in (0, 1):
                uf = sp.tile([128, CW2], F32, name=f"uf{hh}", tag=f"uf{hh}")
                nc.sync.dma_start(out=uf[:].rearrange("p (T c) -> p T c", c=C),
                                  in_=bass.AP(logf_d[:].tensor, hh * NTH * CST * C,
                                              [[C, 128], [CST * C, NTH], [1, C]]))
                uf_h[hh] = uf

            # ---- reverse bwd logits lanes (R @ logitB), then to DRAM ----
            with tc.tile_pool(name="prev", bufs=1, space="PSUM") as pr:
                psr = pr.tile([128, ST * C], F32, name="psrev")
                nc.tensor.matmul(psr[:, 0:512], lhsT=r_sb[:], rhs=logit_sb["b"][:, 0:512],
                                 start=True, stop=True)
                nc.tensor.matmul(psr[:, 512:ST * C], lhsT=r_sb[:],
                                 rhs=logit_sb["b"][:, 512:ST * C], start=True, stop=True)
                lrev = sp.tile([128, ST * C], F32, name="lrev", tag="lrev")
                nc.vector.tensor_copy(lrev[:], psr[:])
                nc.sync.dma_start(
                    out=logb_d[0:WINW, :].rearrange("(l s) c -> l (s c)", s=ST),
                    in_=lrev[:],
                )

            # ---- CRF ----
            with (
                tc.tile_pool(name="crf", bufs=2) as fp,
                tc.tile_pool(name="crfc", bufs=1) as fc1,
                tc.tile_pool(name="psc", bufs=2, space="PSUM") as pc,
            ):
                shi_sb = fc1.tile([128, 128], BF16, name="shi")
                nc.sync.dma_start(out=shi_sb[:], in_=shi[:])
                slo_sb = fc1.tile([128, 128], BF16, name="slo")
                nc.sync.dma_start(out=slo_sb[:], in_=slo[:])
                valid_sb = fc1.tile([128, NT], F32, name="valid")
                nc.sync.dma_start(out=valid_sb[:], in_=valid[:])

                # two independent half-window pipelines (T 0..13 / 14..27) so
                # the serial softmax chain of one half overlaps the other's
                # matmuls across engines
                u_h, xcur = {}, {}
                for hh in (0, 1):
                    ub = fc1.tile([128, CW2], F32, name=f"ub{hh}")
                    nc.sync.dma_start(out=ub[:].rearrange("p (T c) -> p T c", c=C),
                                      in_=bass.AP(logb_d[:].tensor,
                                                  64 * C + hh * NTH * CST * C,
                                                  [[C, 128], [CST * C, NTH], [1, C]]))
                    u = fc1.tile([128, CW2], F32, name=f"u{hh}")
                    nc.vector.tensor_add(u[:], uf_h[hh][:], ub[:])
                    u_h[hh] = u
                    xcur[hh] = u
                for it in range(NIT + 1):
                    last = it == NIT
                    e_h = {}
                    for hh in (0, 1):
                        e = fp.tile([128, CW2], F32, name=f"e{it}{hh}", tag=f"e{hh}")
                        nc.scalar.activation(e[:], xcur[hh][:], AF.Exp)
                        e_h[hh] = e
                    for hh in (0, 1):
                        e = e_h[hh]
                        ssum = fp.tile([128, NTH], F32, name=f"ss{it}{hh}", tag=f"ss{hh}")
                        nc.vector.reduce_sum(ssum[:],
                                             e[:].rearrange("p (T c) -> p T c", c=C),
                                             axis=mybir.AxisListType.X)
                        rv = fp.tile([128, NTH], F32, name=f"rv{it}{hh}", tag=f"rv{hh}")
                        nc.vector.reciprocal(rv[:], ssum[:])
                        if not last:
                            rvv = fp.tile([128, NTH], F32, name=f"rvv{it}{hh}", tag=f"rvv{hh}")
                            nc.vector.tensor_mul(rvv[:], rv[:],
                                                 valid_sb[:, hh * NTH:(hh + 1) * NTH])
                            p = fp.tile([128, CW2], BF16, name=f"p{it}{hh}", tag=f"p{hh}")
                            nc.vector.tensor_mul(
                                p[:].rearrange("p (T c) -> p T c", c=C),
                                e[:].rearrange("p (T c) -> p T c", c=C),
                                rvv[:].unsqueeze(2).broadcast_to([128, NTH, C]))
                            psc = pc.tile([128, CW2], F32, name=f"pc{it}{hh}", tag=f"pc{hh}")
                            nc.tensor.matmul(psc[:], lhsT=shi_sb[:], rhs=p[:],
                                             start=True, stop=False)
                            nc.tensor.matmul(psc[:], lhsT=slo_sb[:], rhs=p[:],
                                             start=False, stop=True)
                            xn = fp.tile([128, CW2], F32, name=f"x{it}{hh}", tag=f"x{hh}")
                            nc.vector.tensor_add(xn[:], u_h[hh][:], psc[:])
                            xcur[hh] = xn
                        else:
                            pout = fp.tile([128, CW2], F32, name=f"pout{hh}", tag=f"p{hh}")
                            nc.vector.tensor_mul(
                                pout[:].rearrange("p (T c) -> p T c", c=C),
                                e[:].rearrange("p (T c) -> p T c", c=C),
                                rv[:].unsqueeze(2).broadcast_to([128, NTH, C]))
                            nc.sync.dma_start(
                                out=bass.AP(out[:].tensor,
                                            25 * C + hh * NTH * CST * C,
                                            [[C, CST], [CST * C, NTH], [1, C]]),
                                in_=pout[25:25 + CST, :].rearrange("p (T c) -> p T c", c=C))

    nc.compile()
    return nc


def _prep(inputs):
    I = {k: np.asarray(v, np.float32) for k, v in inputs.items()}
    x = I["batch"]
    xr = x[::-1]
    bf = ml_dtypes.bfloat16
    f8 = ml_dtypes.float8_e4m3

    Wihf = I["W_ih_f"].T          # (512, 2048)
    Wihb = I["W_ih_b"].T
    Whhf = I["W_hh_f"].T          # (512, 2048)
    Whhb = I["W_hh_b"].T
    biasf = (I["b_ih_f"] + I["b_hh_f"])[None, :]
    biasb = (I["b_ih_b"] + I["b_hh_b"])[None, :]
    WlinT = I["W_lin"].T          # (1024, 32)

    half = FILT // 2
    dd = np.arange(-half, half + 1, dtype=np.float32)
    kern = np.exp(-(dd * I["inv_smoothness_theta"][0]) ** 2 / 2)
    kern[half] = 0.0
    kern *= I["smoothness_weight"]
    S = np.zeros((128, 128), np.float32)
    for i in range(128):
        for j in range(max(0, i - half), min(128, i + half + 1)):
            if i != j:
                S[i, j] = kern[j - i + half]
    S_hi = S.astype(bf).astype(np.float32)
    S_lo = (S - S_hi).astype(bf)

    shared = dict(
        wihf=Wihf.reshape(4, 128, G).astype(bf),
        wihb=Wihb.reshape(4, 128, G).astype(bf),
        whhf8=Whhf.reshape(4, 128, G).astype(f8),
        whhb8=Whhb.reshape(4, 128, G).astype(f8),
        biasf=biasf.astype(bf), biasb=biasb.astype(bf),
        wlinf=WlinT[:512].reshape(4, 128, C).astype(bf),
        wlinb=WlinT[512:].reshape(4, 128, C).astype(bf),
        blin=I["b_lin"][None, :].astype(bf),
        ones=np.ones((1, 128), bf),
        ident=np.eye(128, dtype=np.float32).astype(bf),
        rmat=np.eye(128, dtype=np.float32)[::-1].copy(),
        shi=S_hi.astype(bf), slo=S_lo,
    )

    def window(src, W0):
        w = np.zeros((K + WINW, EMB), np.float32)
        lo, hi = W0 - K, W0 + WINW
        slo, shi_ = max(lo, 0), min(hi, SEQ)
        if shi_ > slo:
            w[slo - lo:shi_ - lo] = src[slo:shi_]
        return np.ascontiguousarray(w.T).reshape(4, 128, K + WINW).astype(bf)

    st = np.arange(STEPS)[:, None]      # (STEPS, 1)
    ll = np.arange(NL)[None, :] * ST    # (1, NL)
    pp = np.arange(128)[:, None]
    TT = np.arange(NT)[None, :] * CST
    in_maps = []
    for c in range(NCORES):
        Wc = 2048 * c - 32
        Wr = 2048 * (7 - c) - 32
        gpos = Wc + TT + pp
        m = dict(shared)
        m["xtf"] = window(x, Wc)
        m["xtb"] = window(xr, Wr)
        # mask[t, lane] = 1 if that (lane, step) reads a position >= 0;
        # it is the lhsT of the bias matmul, so masked lanes get zero bias
        # and (with zeroed x and h) stay exactly at zero state.
        m["maskf"] = ((ll + st + Wc - K) >= 0).astype(bf).reshape(1, STEPS * NL)
        m["maskb"] = ((ll + st + Wr - K) >= 0).astype(bf).reshape(1, STEPS * NL)
        m["valid"] = ((gpos >= 0) & (gpos < SEQ) & (TT + pp < WINW)).astype(np.float32)
        in_maps.append(m)
    return in_maps


def _run(inputs, trace=False, trace_cores=None):
    if "nc" not in _CACHE:
        _CACHE["nc"] = _build()
    nc = _CACHE["nc"]
    in_maps = _prep(inputs)
    kw = {}
    if trace:
        import types
        try:
            import trn_agent_boot.trn_boot as tb
            hook = tb._ntff_profile_via_ctypes("/opt/axon/libaxon_pjrt.so")
            mod = types.ModuleType("antenv.axon_hooks")
            mod.get_axon_ntff_profile_hook = lambda: hook
            sys.modules.setdefault("antenv.axon_hooks", mod)
        except Exception:
            pass
        kw = dict(trace=True, trace_cores=trace_cores or list(range(NCORES)))
    res = run_bass_kernel_spmd(nc, in_maps, list(range(NCORES)), **kw)
    full = np.zeros((SEQ, C), np.float32)
    for c in range(NCORES):
        full[2048 * c:2048 * (c + 1)] = res.results[c]["out"][32:2080]
    return full, res


def kernel(**inputs):
    full, _ = _run(inputs)
    return full
